# revision 2
# baseline (speedup 1.0000x reference)
"""Multi-head attention (B=8, N=1024, D=768, H=12) on 8 Trainium2 NeuronCores.

Sharding: pure data parallel — one batch element per core, weights replicated,
no collectives. Host-side prep (part of this kernel): x pre-transposed + fp16;
w_qkv's Q/K columns reordered into per-head-pair blocks [q0,k0,q1,k1,...];
w_qkv's V columns augmented to 65-wide per-head blocks [v_h | 0] whose last
column is all-zeros with bias 1.0 — the AV matmul (M=65) then emits the
softmax denominator in PSUM row 64 for free (no M=1 sum matmuls).

Structure (vs the 237µs baseline; measured ~189µs):
  - PE-HAM warmup via accumulation-chain dummies (isolated matmuls don't trip
    the activity monitor) paced across the input-DMA window, so real work
    runs at the warm 2.4 GHz clock from the start.
  - The ScalarE exp chain (~100µs serial floor) is the attention bottleneck:
    the first two slots' score/exp units are emitted during the v phase
    (deep pt ring; their AV matmuls follow later), and subsequent qkT
    head-pair chunks are injected between attention units to soak up the PE
    slack instead of stalling the exp pipeline at slot boundaries.
  - All K=1 bias matmuls replaced by bias tiles (GpSimd partition_broadcast)
    folded into the PSUM->SBUF copies; reciprocal-broadcast also on GpSimd.
  - Projection pipelined in t=0..4 / t=5 steps so the last head pair's
    normalize tail overlaps projection work; trailing dummy chains keep the
    clock gate warm across the pool-scope boundary.

Per-core dataflow (feature-major, stationary operands K-major):
    v [n,c]   = x @ wv_aug (+bias via tile)   (65-wide head blocks)
    qkT[c,n]  = wqk blocks .T-contract        (lhsT = wqk block, rhs = xT)
    sT [m,n]  = k_h qT_h                      (row-packed K=64 head pair)
    pT        = exp(SCALE * sT)               (ScalarE)
    av_h      = [v_h|1].T @ pT                (M=65: rows 0:64 = wa, 64 = sum)
    waT       = av[0:64] * bcast(1/av[64])
    out[n,c]  = waT.T @ w_proj (+bias tile)
"""

import numpy as np

import concourse.bass as bass
import concourse.bacc as bacc
import concourse.tile as tile
from concourse import mybir
from concourse.bass_utils import run_bass_kernel_spmd

F32 = mybir.dt.float32
F16 = mybir.dt.float16
EXP = mybir.ActivationFunctionType.Exp

B = 8
N = 1024
D = 768
H = 12
HD = 64
SCALE = HD ** -0.5
NT = N // 128       # 8 tiles along sequence
DT = D // 128       # 6 tiles along features
NPAIR = H // 2      # 6 head pairs
VW = H * (HD + 1)   # 780: augmented V width (64 values + 1 ones col per head)
EARLY = 16          # units (2 slots) whose score/exp are emitted in phase V
PTBUFS = EARLY + 2


def build_nc() -> bass.Bass:
    nc = bacc.Bacc(None)
    xT_d = nc.dram_tensor("xT", [D, N], F16, kind="ExternalInput")
    wqk_d = nc.dram_tensor("wqk", [D, 2 * D], F16, kind="ExternalInput")
    wv_d = nc.dram_tensor("wv", [D, VW], F16, kind="ExternalInput")
    bqk_d = nc.dram_tensor("bqk", [2 * D], F32, kind="ExternalInput")
    bv_d = nc.dram_tensor("bv", [VW], F32, kind="ExternalInput")
    wproj_d = nc.dram_tensor("w_proj", [D, D], F16, kind="ExternalInput")
    bproj_d = nc.dram_tensor("b_proj", [D], F32, kind="ExternalInput")
    out_d = nc.dram_tensor("out", [N, D], F16, kind="ExternalOutput")

    with tile.TileContext(nc) as tc:
        _emit(nc, tc, xT_d, wqk_d, wv_d, bqk_d, bv_d, wproj_d, bproj_d, out_d)
    nc.compile()
    return nc


def _emit(nc, tc, xT_d, wqk_d, wv_d, bqk_d, bv_d, wproj_d, bproj_d, out_d):
    from contextlib import ExitStack

    with ExitStack() as ctx:
        const = ctx.enter_context(tc.tile_pool(name="const", bufs=1))
        qkv_pool = ctx.enter_context(tc.tile_pool(name="qkv", bufs=1))
        ptp = ctx.enter_context(tc.tile_pool(name="ptile", bufs=PTBUFS))
        rsp = ctx.enter_context(tc.tile_pool(name="rsp", bufs=2))
        # pst+pqk span phases V and attention, released before proj
        qk_ctx = tc.tile_pool(name="pqk", bufs=2, space="PSUM")
        st_ctx = tc.tile_pool(name="pst", bufs=2, space="PSUM")
        pst = st_ctx.__enter__()
        pqk = qk_ctx.__enter__()

        bqk_col = const.tile([128, 2 * NPAIR], F32, tag="bqk_col")
        nc.sync.dma_start(
            out=bqk_col, in_=bqk_d.rearrange("(b p) -> p b", p=128)
        )
        bv32 = const.tile([1, VW], F32, tag="bv32")
        nc.sync.dma_start(out=bv32, in_=bv_d[None, :])
        bp32 = const.tile([1, D], F32, tag="bp32")
        nc.sync.dma_start(out=bp32, in_=bproj_d[None, :])
        bv_tile = const.tile([128, VW], F32, tag="bv_tile")
        nc.gpsimd.partition_broadcast(bv_tile, bv32, channels=128)
        bp_tile = const.tile([128, D], F32, tag="bp_tile")
        nc.gpsimd.partition_broadcast(bp_tile, bp32, channels=128)
        zbias = const.tile([128, 1], F32, tag="zbias")
        nc.vector.memset(zbias, 0.0)
        warm = const.tile([128, 512], F16, tag="warm")
        nc.vector.memset(warm, 0.5)

        xT = [qkv_pool.tile([128, N], F16, tag=f"xT{t}", name=f"xT{t}")
              for t in range(DT)]
        wv_all = qkv_pool.tile([128, DT * VW], F16, tag="wv_all")
        wqk_all = qkv_pool.tile([128, 2 * D * DT], F16, tag="wqk_all")
        wproj_all = qkv_pool.tile([128, D * DT], F16, tag="wproj_all")

        for h in range(2):
            for t in range(DT):
                nc.sync.dma_start(
                    out=xT[t][:, 512 * h:512 * (h + 1)],
                    in_=xT_d[128 * t:128 * (t + 1), 512 * h:512 * (h + 1)],
                )
        for g in range(2):
            t0, t1 = 3 * g, 3 * (g + 1)
            nc.sync.dma_start(
                out=wv_all[:, t0 * VW:t1 * VW].rearrange(
                    "p (t c) -> p t c", c=VW),
                in_=wv_d[128 * t0:128 * t1, :].rearrange(
                    "(t p) c -> p t c", p=128),
            )
        for b in range(2 * NPAIR):
            nc.sync.dma_start(
                out=wqk_all[:, b * DT * 128:(b + 1) * DT * 128].rearrange(
                    "p (t c) -> p t c", c=128),
                in_=wqk_d[:, 128 * b:128 * (b + 1)].rearrange(
                    "(t p) c -> p t c", p=128),
            )
        for g in range(2):
            t0, t1 = 3 * g, 3 * (g + 1)
            nc.sync.dma_start(
                out=wproj_all[:, t0 * D:t1 * D].rearrange(
                    "p (t c) -> p t c", c=D),
                in_=wproj_d[128 * t0:128 * t1, :].rearrange(
                    "(t p) c -> p t c", p=128),
            )

        qkT = [qkv_pool.tile([128, N], F16, tag=f"qkT{j}", name=f"qkT{j}")
               for j in range(2 * NPAIR)]
        v_sb = [qkv_pool.tile([128, VW], F16, tag=f"v{i}", name=f"v{i}")
                for i in range(NT)]
        waT = [qkv_pool.tile([128, N], F16, tag=f"waT{p}", name=f"waT{p}")
               for p in range(NPAIR)]

        def dummy_chain(n, lhsT, rhs):
            dum = pqk.tile([128, 512], F32, tag="qk", name="qk")
            for d in range(n):
                nc.tensor.matmul(dum, lhsT, rhs, start=(d == 0),
                                 stop=(d == n - 1))

        def emit_qk_chunk(p, half, nch):
            """One qkT chunk: half=0 -> q-tile p (block 2p), half=1 ->
            k-tile NPAIR+p (block 2p+1)."""
            j = p if half == 0 else NPAIR + p
            b = 2 * p + half
            ps = pqk.tile([128, 512], F32, tag="qk", name="qk")
            for t in range(DT):
                nc.tensor.matmul(
                    ps,
                    wqk_all[:, (b * DT + t) * 128:(b * DT + t + 1) * 128],
                    xT[t][:, 512 * nch:512 * (nch + 1)],
                    start=(t == 0), stop=(t == DT - 1),
                )
            nc.vector.tensor_scalar_add(
                qkT[j][:, 512 * nch:512 * (nch + 1)], ps, bqk_col[:, b:b + 1]
            )

        def emit_st(p, nch, m):
            """Row-packed K=64 pair: scores^T for heads (2p, 2p+1)."""
            m0, n0 = 128 * m, 512 * nch
            st = pst.tile([128, 1024], F32, tag="st", name="st")
            nc.tensor.matmul(
                st[:, 0:512],
                qkT[NPAIR + p][0:64, m0:m0 + 128],
                qkT[p][0:64, n0:n0 + 512],
                start=True, stop=True,
            )
            nc.tensor.matmul(
                st[:, 512:1024],
                qkT[NPAIR + p][64:128, m0:m0 + 128],
                qkT[p][64:128, n0:n0 + 512],
                start=True, stop=True,
            )
            return st

        def emit_exp(st):
            pt_t = ptp.tile([128, 1024], F16, tag="pt", name="pt")
            nc.scalar.activation(pt_t, st, EXP, bias=zbias, scale=SCALE)
            return pt_t

        # ---- Phase V: v + qkT pair 0 + early score/exp units ----
        early_pts = []
        with tc.tile_pool(name="pv", bufs=1, space="PSUM") as pv:
            # warmup chains paced across the input DMA stream
            dummy_chain(9, warm[:, 0:128], warm)
            for t in range(DT):
                dummy_chain(2, xT[t][:, 0:128], xT[t][:, 0:512])
            dummy_chain(2, xT[5][:, 512:640], xT[5][:, 512:1024])
            dummy_chain(3, wv_all[:, 0:128], wv_all[:, 0:512])
            dummy_chain(2, wqk_all[:, 0:128], wqk_all[:, 0:512])

            # qkT pair 0, nch-major so slot (0,0) scores start first
            for nch in range(2):
                for half in range(2):
                    emit_qk_chunk(0, half, nch)

            def emit_early(k):
                # unit k of slots (0,0) / (0,1): score + exp only
                nch, m = divmod(k, NT)
                early_pts.append(emit_exp(emit_st(0, nch, m)))

            def emit_v(i):
                ps = pv.tile([128, VW], F32, tag="v")
                for c0, cw in ((0, 512), (512, VW - 512)):
                    for t in range(DT):
                        nc.tensor.matmul(
                            ps[:, c0:c0 + cw],
                            xT[t][:, 128 * i:128 * (i + 1)],
                            wv_all[:, t * VW + c0:t * VW + c0 + cw],
                            start=(t == 0), stop=(t == DT - 1),
                        )
                nc.vector.tensor_add(v_sb[i], ps, bv_tile)

            # interleave: early score/exp units + v tiles + pair-1 chunks
            emit_early(0)
            emit_early(1)
            for i in range(NT):
                emit_v(i)
                for k in (2 + 2 * i, 3 + 2 * i):
                    if k < EARLY:
                        emit_early(k)
                if i in (4, 5, 6, 7):
                    emit_qk_chunk(1, (i - 4) % 2, (i - 4) // 2)

        # ---- Attention slots (qkT pair p>=2 chunks injected between) ----
        with tc.tile_pool(name="pav", bufs=1, space="PSUM") as pav:
            # injection schedule: pair p's 4 chunks spread over slots
            # 2p-3, 2p-2 (2 chunks per slot, after units 2 and 5)
            inject = {}
            for p in range(2, NPAIR):
                inject[(2 * p - 3, 2)] = (p, 0, 0)
                inject[(2 * p - 3, 5)] = (p, 1, 0)
                inject[(2 * p - 2, 2)] = (p, 0, 1)
                inject[(2 * p - 2, 5)] = (p, 1, 1)

            slots = [(p, nch) for p in range(NPAIR) for nch in range(2)]
            st_next = None
            for si, (p, nch) in enumerate(slots):
                hA, hB = 2 * p, 2 * p + 1
                n0 = 512 * nch
                av = pav.tile([128, 1024], F32, tag="av", name="av")
                for m in range(NT):
                    if si >= 2:
                        st = st_next if st_next is not None \
                            else emit_st(p, nch, m)
                        st_next = None
                        pt_t = emit_exp(st)
                    else:
                        pt_t = early_pts[si * NT + m]
                    if (si, m) in inject:
                        ip, ihalf, inch = inject[(si, m)]
                        emit_qk_chunk(ip, ihalf, inch)
                    # pre-emit the next unit's score pair
                    if si >= 1 and not (si == len(slots) - 1 and m == NT - 1):
                        nsi, nm = (si, m + 1) if m < NT - 1 else (si + 1, 0)
                        if nsi >= 2:
                            np_, nnch = slots[nsi]
                            st_next = emit_st(np_, nnch, nm)
                    nc.tensor.matmul(
                        av[0:65, 0:512],
                        v_sb[m][:, 65 * hA:65 * hA + 65],
                        pt_t[:, 0:512],
                        start=(m == 0), stop=(m == NT - 1),
                    )
                    nc.tensor.matmul(
                        av[0:65, 512:1024],
                        v_sb[m][:, 65 * hB:65 * hB + 65],
                        pt_t[:, 512:1024],
                        start=(m == 0), stop=(m == NT - 1),
                    )
                # normalize: sums+values to SBUF (frees av banks early),
                # reciprocal, GpSimd broadcast, two scaling multiplies
                sm = rsp.tile([1, 1024], F32, tag="sm", name="sm")
                nc.vector.tensor_copy(sm, av[64:65, :])
                avs = rsp.tile([64, 1024], F32, tag="avs", name="avs")
                nc.vector.tensor_copy(avs, av[0:64, :])
                ra = rsp.tile([1, 1024], F32, tag="ra", name="ra")
                nc.vector.reciprocal_approx_fast(ra, sm)
                bc = rsp.tile([64, 1024], F32, tag="bc", name="bc")
                nc.gpsimd.partition_broadcast(bc, ra, channels=64)
                nc.vector.tensor_mul(waT[p][0:64, n0:n0 + 512],
                                     avs[:, 0:512], bc[:, 0:512])
                nc.vector.tensor_mul(waT[p][64:128, n0:n0 + 512],
                                     avs[:, 512:1024], bc[:, 512:1024])

            # trailing dummy chains bridge the pool boundary into proj
            for c in range(2):
                dummy_chain(6, warm[:, 0:128], warm)
        qk_ctx.__exit__(None, None, None)
        st_ctx.__exit__(None, None, None)

        # ---- Output projection: A = t0..4 accumulation, B = t5 + store ----
        with tc.tile_pool(name="po", bufs=3, space="PSUM") as po, \
             tc.tile_pool(name="ob", bufs=3) as obp:
            def proj_a(i):
                ps = po.tile([128, D], F32, tag="o", name="o")
                for c0, cw in ((0, 512), (512, 256)):
                    for t in range(DT - 1):
                        nc.tensor.matmul(
                            ps[:, c0:c0 + cw],
                            waT[t][:, 128 * i:128 * (i + 1)],
                            wproj_all[:, t * D + c0:t * D + c0 + cw],
                            start=(t == 0), stop=False,
                        )
                return ps

            def proj_b(i, ps):
                t = DT - 1
                for c0, cw in ((0, 512), (512, 256)):
                    nc.tensor.matmul(
                        ps[:, c0:c0 + cw],
                        waT[t][:, 128 * i:128 * (i + 1)],
                        wproj_all[:, t * D + c0:t * D + c0 + cw],
                        start=False, stop=True,
                    )
                ot = obp.tile([128, D], F16, tag="ot", name="ot")
                nc.vector.tensor_add(ot, ps, bp_tile)
                nc.sync.dma_start(out=out_d[128 * i:128 * (i + 1), :], in_=ot)

            pend = []
            for i in range(NT):
                pend.append((i, proj_a(i)))
                if len(pend) == 3:
                    proj_b(*pend.pop(0))
            for i, ps in pend:
                proj_b(i, ps)


def _prep_host(inputs):
    """Host-side weight reorder/augment (shared across cores)."""
    w_qkv = np.asarray(inputs["w_qkv"], dtype=np.float32)
    b_qkv = np.asarray(inputs["b_qkv"], dtype=np.float32)
    wqk = np.empty((D, 2 * D), dtype=np.float16)
    bqk = np.empty((2 * D,), dtype=np.float32)
    for p2 in range(NPAIR):
        wqk[:, 256 * p2:256 * p2 + 128] = w_qkv[:, 128 * p2:128 * (p2 + 1)]
        wqk[:, 256 * p2 + 128:256 * p2 + 256] = \
            w_qkv[:, D + 128 * p2:D + 128 * (p2 + 1)]
        bqk[256 * p2:256 * p2 + 128] = b_qkv[128 * p2:128 * (p2 + 1)]
        bqk[256 * p2 + 128:256 * p2 + 256] = \
            b_qkv[D + 128 * p2:D + 128 * (p2 + 1)]
    wv = np.zeros((D, VW), dtype=np.float16)
    bv = np.zeros((VW,), dtype=np.float32)
    for h in range(H):
        wv[:, 65 * h:65 * h + 64] = \
            w_qkv[:, 2 * D + 64 * h:2 * D + 64 * h + 64]
        bv[65 * h:65 * h + 64] = b_qkv[2 * D + 64 * h:2 * D + 64 * h + 64]
        bv[65 * h + 64] = 1.0
    return {
        "wqk": wqk,
        "bqk": bqk,
        "wv": wv,
        "bv": bv,
        "w_proj": np.asarray(inputs["w_proj"], dtype=np.float32).astype(np.float16),
        "b_proj": np.asarray(inputs["b_proj"], dtype=np.float32),
    }


def run(inputs: dict, trace: bool = False):
    """Build, compile and run on all 8 cores. Returns (out [B,N,D], results)."""
    nc = build_nc()
    x = np.asarray(inputs["x"], dtype=np.float32)
    shared = _prep_host(inputs)
    in_maps = [
        {"xT": np.ascontiguousarray(x[b].T.astype(np.float16)), **shared}
        for b in range(B)
    ]
    res = run_bass_kernel_spmd(nc, in_maps, list(range(B)), trace=trace)
    out = np.stack(
        [res.results[b]["out"].astype(np.float32) for b in range(B)], axis=0
    )
    return out, res


def kernel(x, w_qkv, b_qkv, w_proj, b_proj) -> np.ndarray:
    out, _ = run(
        {"x": x, "w_qkv": w_qkv, "b_qkv": b_qkv, "w_proj": w_proj,
         "b_proj": b_proj}
    )
    return out


# revision 3
# speedup vs baseline: 1.1800x; 1.1800x over previous
"""Multi-head attention (B=8, N=1024, D=768, H=12) on 8 Trainium2 NeuronCores.

Sharding: pure data parallel — one batch element per core, weights replicated,
no collectives. Host-side prep (part of this kernel): x pre-transposed + fp16;
w_qkv's Q/K columns reordered into per-head-pair blocks [q0,k0,q1,k1,...];
w_qkv's V columns augmented to 65-wide per-head blocks [v_h | 0] whose last
column is all-zeros with bias 1.0 — the AV matmul (M=65) then emits the
softmax denominator in PSUM row 64 for free (no M=1 sum matmuls).

Structure (vs the 237µs baseline; ~189µs at the normal 2.4 GHz clock):
  - PE-HAM warmup via accumulation-chain dummies (isolated matmuls don't trip
    the activity monitor) paced across the input-DMA window, so real work
    runs at the warm 2.4 GHz clock from the start.
  - The ScalarE exp chain (~100µs serial floor) is the attention bottleneck:
    the first two slots' score/exp units are emitted during the v phase
    (deep pt ring; their AV matmuls follow later), and subsequent qkT
    head-pair chunks are injected between attention units to soak up the PE
    slack instead of stalling the exp pipeline at slot boundaries.
  - All K=1 bias matmuls replaced by bias tiles (GpSimd partition_broadcast)
    folded into the PSUM->SBUF copies; reciprocal-broadcast also on GpSimd.
  - Projection pipelined in t=0..4 / t=5 steps so the last head pair's
    normalize tail overlaps projection work; trailing dummy chains keep the
    clock gate warm across the pool-scope boundary.

Per-core dataflow (feature-major, stationary operands K-major):
    v [n,c]   = x @ wv_aug (+bias via tile)   (65-wide head blocks)
    qkT[c,n]  = wqk blocks .T-contract        (lhsT = wqk block, rhs = xT)
    sT [m,n]  = k_h qT_h                      (row-packed K=64 head pair)
    pT        = exp(SCALE * sT)               (ScalarE)
    av_h      = [v_h|1].T @ pT                (M=65: rows 0:64 = wa, 64 = sum)
    waT       = av[0:64] * bcast(1/av[64])
    out[n,c]  = waT.T @ w_proj (+bias tile)
"""

import numpy as np

import concourse.bass as bass
import concourse.bacc as bacc
import concourse.tile as tile
from concourse import mybir
from concourse.bass_utils import run_bass_kernel_spmd

F32 = mybir.dt.float32
F16 = mybir.dt.float16
EXP = mybir.ActivationFunctionType.Exp

B = 8
N = 1024
D = 768
H = 12
HD = 64
SCALE = HD ** -0.5
NT = N // 128       # 8 tiles along sequence
DT = D // 128       # 6 tiles along features
NPAIR = H // 2      # 6 head pairs
VW = H * (HD + 1)   # 780: augmented V width (64 values + 1 ones col per head)
EARLY = 16          # units (2 slots) whose score/exp are emitted in phase V
PTBUFS = EARLY + 2


def build_nc() -> bass.Bass:
    nc = bacc.Bacc(None)
    xT_d = nc.dram_tensor("xT", [D, N], F16, kind="ExternalInput")
    wqk_d = nc.dram_tensor("wqk", [D, 2 * D], F16, kind="ExternalInput")
    wv_d = nc.dram_tensor("wv", [D, VW], F16, kind="ExternalInput")
    bqk_d = nc.dram_tensor("bqk", [2 * D], F32, kind="ExternalInput")
    bv_d = nc.dram_tensor("bv", [VW], F32, kind="ExternalInput")
    wproj_d = nc.dram_tensor("w_proj", [D, D], F16, kind="ExternalInput")
    bproj_d = nc.dram_tensor("b_proj", [D], F32, kind="ExternalInput")
    out_d = nc.dram_tensor("out", [N, D], F16, kind="ExternalOutput")

    with tile.TileContext(nc) as tc:
        _emit(nc, tc, xT_d, wqk_d, wv_d, bqk_d, bv_d, wproj_d, bproj_d, out_d)
    nc.compile()
    return nc


def _emit(nc, tc, xT_d, wqk_d, wv_d, bqk_d, bv_d, wproj_d, bproj_d, out_d):
    from contextlib import ExitStack

    with ExitStack() as ctx:
        const = ctx.enter_context(tc.tile_pool(name="const", bufs=1))
        qkv_pool = ctx.enter_context(tc.tile_pool(name="qkv", bufs=1))
        ptp = ctx.enter_context(tc.tile_pool(name="ptile", bufs=PTBUFS))
        rsp = ctx.enter_context(tc.tile_pool(name="rsp", bufs=2))
        # pst+pqk span phases V and attention, released before proj
        qk_ctx = tc.tile_pool(name="pqk", bufs=2, space="PSUM")
        st_ctx = tc.tile_pool(name="pst", bufs=2, space="PSUM")
        pst = st_ctx.__enter__()
        pqk = qk_ctx.__enter__()

        bqk_col = const.tile([128, 2 * NPAIR], F32, tag="bqk_col")
        nc.sync.dma_start(
            out=bqk_col, in_=bqk_d.rearrange("(b p) -> p b", p=128)
        )
        bv32 = const.tile([1, VW], F32, tag="bv32")
        nc.sync.dma_start(out=bv32, in_=bv_d[None, :])
        bp32 = const.tile([1, D], F32, tag="bp32")
        nc.sync.dma_start(out=bp32, in_=bproj_d[None, :])
        bv_tile = const.tile([128, VW], F32, tag="bv_tile")
        nc.gpsimd.partition_broadcast(bv_tile, bv32, channels=128)
        bp_tile = const.tile([128, D], F32, tag="bp_tile")
        nc.gpsimd.partition_broadcast(bp_tile, bp32, channels=128)
        zbias = const.tile([128, 1], F32, tag="zbias")
        nc.vector.memset(zbias, 0.0)
        warm = const.tile([128, 512], F16, tag="warm")
        nc.vector.memset(warm, 0.5)

        xT = [qkv_pool.tile([128, N], F16, tag=f"xT{t}", name=f"xT{t}")
              for t in range(DT)]
        wv_all = qkv_pool.tile([128, DT * VW], F16, tag="wv_all")
        wqk_all = qkv_pool.tile([128, 2 * D * DT], F16, tag="wqk_all")
        wproj_all = qkv_pool.tile([128, D * DT], F16, tag="wproj_all")

        for h in range(2):
            for t in range(DT):
                nc.sync.dma_start(
                    out=xT[t][:, 512 * h:512 * (h + 1)],
                    in_=xT_d[128 * t:128 * (t + 1), 512 * h:512 * (h + 1)],
                )
        for g in range(2):
            t0, t1 = 3 * g, 3 * (g + 1)
            nc.sync.dma_start(
                out=wv_all[:, t0 * VW:t1 * VW].rearrange(
                    "p (t c) -> p t c", c=VW),
                in_=wv_d[128 * t0:128 * t1, :].rearrange(
                    "(t p) c -> p t c", p=128),
            )
        for b in range(2 * NPAIR):
            nc.sync.dma_start(
                out=wqk_all[:, b * DT * 128:(b + 1) * DT * 128].rearrange(
                    "p (t c) -> p t c", c=128),
                in_=wqk_d[:, 128 * b:128 * (b + 1)].rearrange(
                    "(t p) c -> p t c", p=128),
            )
        for g in range(2):
            t0, t1 = 3 * g, 3 * (g + 1)
            nc.sync.dma_start(
                out=wproj_all[:, t0 * D:t1 * D].rearrange(
                    "p (t c) -> p t c", c=D),
                in_=wproj_d[128 * t0:128 * t1, :].rearrange(
                    "(t p) c -> p t c", p=128),
            )

        qkT = [qkv_pool.tile([128, N], F16, tag=f"qkT{j}", name=f"qkT{j}")
               for j in range(2 * NPAIR)]
        v_sb = [qkv_pool.tile([128, VW], F16, tag=f"v{i}", name=f"v{i}")
                for i in range(NT)]
        waT = [qkv_pool.tile([128, N], F16, tag=f"waT{p}", name=f"waT{p}")
               for p in range(NPAIR)]

        def dummy_chain(n, lhsT, rhs):
            dum = pqk.tile([128, 512], F32, tag="qk", name="qk")
            for d in range(n):
                nc.tensor.matmul(dum, lhsT, rhs, start=(d == 0),
                                 stop=(d == n - 1))

        def emit_qk_chunk(p, half, nch):
            """One qkT chunk: half=0 -> q-tile p (block 2p), half=1 ->
            k-tile NPAIR+p (block 2p+1)."""
            j = p if half == 0 else NPAIR + p
            b = 2 * p + half
            ps = pqk.tile([128, 512], F32, tag="qk", name="qk")
            for t in range(DT):
                nc.tensor.matmul(
                    ps,
                    wqk_all[:, (b * DT + t) * 128:(b * DT + t + 1) * 128],
                    xT[t][:, 512 * nch:512 * (nch + 1)],
                    start=(t == 0), stop=(t == DT - 1),
                )
            nc.vector.tensor_scalar_add(
                qkT[j][:, 512 * nch:512 * (nch + 1)], ps, bqk_col[:, b:b + 1]
            )

        def emit_st(p, nch, m):
            """Row-packed K=64 pair: scores^T for heads (2p, 2p+1)."""
            m0, n0 = 128 * m, 512 * nch
            st = pst.tile([128, 1024], F32, tag="st", name="st")
            nc.tensor.matmul(
                st[:, 0:512],
                qkT[NPAIR + p][0:64, m0:m0 + 128],
                qkT[p][0:64, n0:n0 + 512],
                start=True, stop=True,
            )
            nc.tensor.matmul(
                st[:, 512:1024],
                qkT[NPAIR + p][64:128, m0:m0 + 128],
                qkT[p][64:128, n0:n0 + 512],
                start=True, stop=True,
            )
            return st

        def emit_exp(st):
            pt_t = ptp.tile([128, 1024], F16, tag="pt", name="pt")
            nc.scalar.activation(pt_t, st, EXP, bias=zbias, scale=SCALE)
            return pt_t

        # ---- Phase V: v + qkT pair 0 + early score/exp units ----
        early_pts = []
        with tc.tile_pool(name="pv", bufs=1, space="PSUM") as pv:
            # warmup chains paced across the input DMA stream
            dummy_chain(9, warm[:, 0:128], warm)
            for t in range(DT):
                dummy_chain(2, xT[t][:, 0:128], xT[t][:, 0:512])
            dummy_chain(2, xT[5][:, 512:640], xT[5][:, 512:1024])
            dummy_chain(3, wv_all[:, 0:128], wv_all[:, 0:512])
            dummy_chain(2, wqk_all[:, 0:128], wqk_all[:, 0:512])

            # qkT pair 0, nch-major so slot (0,0) scores start first
            for nch in range(2):
                for half in range(2):
                    emit_qk_chunk(0, half, nch)

            def emit_early(k):
                # unit k of slots (0,0) / (0,1): score + exp only
                nch, m = divmod(k, NT)
                early_pts.append(emit_exp(emit_st(0, nch, m)))

            def emit_v(i):
                ps = pv.tile([128, VW], F32, tag="v")
                for c0, cw in ((0, 512), (512, VW - 512)):
                    for t in range(DT):
                        nc.tensor.matmul(
                            ps[:, c0:c0 + cw],
                            xT[t][:, 128 * i:128 * (i + 1)],
                            wv_all[:, t * VW + c0:t * VW + c0 + cw],
                            start=(t == 0), stop=(t == DT - 1),
                        )
                nc.vector.tensor_add(v_sb[i], ps, bv_tile)

            # interleave: early score/exp units + v tiles + pair-1 chunks
            emit_early(0)
            emit_early(1)
            for i in range(NT):
                emit_v(i)
                for k in (2 + 2 * i, 3 + 2 * i):
                    if k < EARLY:
                        emit_early(k)
                if i in (4, 5, 6, 7):
                    emit_qk_chunk(1, (i - 4) % 2, (i - 4) // 2)

        # ---- Attention slots (qkT pair p>=2 chunks injected between) ----
        with tc.tile_pool(name="pav", bufs=1, space="PSUM") as pav:
            # injection schedule: pair p's 4 chunks spread over slots
            # 2p-3, 2p-2 (2 chunks per slot, after units 2 and 5)
            inject = {}
            for p in range(2, NPAIR):
                # two consecutive chunks per injection point: halves the
                # pipeline-break (isolated-matmul drain) penalties per pair
                inject[(2 * p - 3, 3)] = ((p, 0, 0), (p, 1, 0))
                inject[(2 * p - 2, 3)] = ((p, 0, 1), (p, 1, 1))

            slots = [(p, nch) for p in range(NPAIR) for nch in range(2)]
            st_next = None
            for si, (p, nch) in enumerate(slots):
                hA, hB = 2 * p, 2 * p + 1
                n0 = 512 * nch
                av = pav.tile([128, 1024], F32, tag="av", name="av")
                for m in range(NT):
                    if si >= 2:
                        st = st_next if st_next is not None \
                            else emit_st(p, nch, m)
                        st_next = None
                        pt_t = emit_exp(st)
                    else:
                        pt_t = early_pts[si * NT + m]
                    if (si, m) in inject:
                        for ip, ihalf, inch in inject[(si, m)]:
                            emit_qk_chunk(ip, ihalf, inch)
                    # pre-emit the next unit's score pair
                    if si >= 1 and not (si == len(slots) - 1 and m == NT - 1):
                        nsi, nm = (si, m + 1) if m < NT - 1 else (si + 1, 0)
                        if nsi >= 2:
                            np_, nnch = slots[nsi]
                            st_next = emit_st(np_, nnch, nm)
                    nc.tensor.matmul(
                        av[0:65, 0:512],
                        v_sb[m][:, 65 * hA:65 * hA + 65],
                        pt_t[:, 0:512],
                        start=(m == 0), stop=(m == NT - 1),
                    )
                    nc.tensor.matmul(
                        av[0:65, 512:1024],
                        v_sb[m][:, 65 * hB:65 * hB + 65],
                        pt_t[:, 512:1024],
                        start=(m == 0), stop=(m == NT - 1),
                    )
                # normalize: sums+values to SBUF (frees av banks early),
                # reciprocal, GpSimd broadcast, two scaling multiplies
                sm = rsp.tile([1, 1024], F32, tag="sm", name="sm")
                nc.vector.tensor_copy(sm, av[64:65, :])
                avs = rsp.tile([64, 1024], F32, tag="avs", name="avs")
                nc.vector.tensor_copy(avs, av[0:64, :])
                ra = rsp.tile([1, 1024], F32, tag="ra", name="ra")
                nc.vector.reciprocal_approx_fast(ra, sm)
                bc = rsp.tile([64, 1024], F32, tag="bc", name="bc")
                nc.gpsimd.partition_broadcast(bc, ra, channels=64)
                nc.vector.tensor_mul(waT[p][0:64, n0:n0 + 512],
                                     avs[:, 0:512], bc[:, 0:512])
                nc.vector.tensor_mul(waT[p][64:128, n0:n0 + 512],
                                     avs[:, 512:1024], bc[:, 512:1024])

            # trailing dummy chains bridge the pool boundary into proj
            for c in range(3):
                dummy_chain(8, warm[:, 0:128], warm)
        qk_ctx.__exit__(None, None, None)
        st_ctx.__exit__(None, None, None)

        # ---- Output projection: A = t0..4 accumulation, B = t5 + store ----
        with tc.tile_pool(name="po", bufs=3, space="PSUM") as po, \
             tc.tile_pool(name="ob", bufs=3) as obp:
            def proj_a(i):
                ps = po.tile([128, D], F32, tag="o", name="o")
                for c0, cw in ((0, 512), (512, 256)):
                    for t in range(DT - 1):
                        nc.tensor.matmul(
                            ps[:, c0:c0 + cw],
                            waT[t][:, 128 * i:128 * (i + 1)],
                            wproj_all[:, t * D + c0:t * D + c0 + cw],
                            start=(t == 0), stop=False,
                        )
                return ps

            def proj_b(i, ps):
                t = DT - 1
                for c0, cw in ((0, 512), (512, 256)):
                    nc.tensor.matmul(
                        ps[:, c0:c0 + cw],
                        waT[t][:, 128 * i:128 * (i + 1)],
                        wproj_all[:, t * D + c0:t * D + c0 + cw],
                        start=False, stop=True,
                    )
                ot = obp.tile([128, D], F16, tag="ot", name="ot")
                nc.vector.tensor_add(ot, ps, bp_tile)
                nc.sync.dma_start(out=out_d[128 * i:128 * (i + 1), :], in_=ot)

            pend = []
            for i in range(NT):
                pend.append((i, proj_a(i)))
                if len(pend) == 3:
                    proj_b(*pend.pop(0))
            for i, ps in pend:
                proj_b(i, ps)


def _prep_host(inputs):
    """Host-side weight reorder/augment (shared across cores)."""
    w_qkv = np.asarray(inputs["w_qkv"], dtype=np.float32)
    b_qkv = np.asarray(inputs["b_qkv"], dtype=np.float32)
    wqk = np.empty((D, 2 * D), dtype=np.float16)
    bqk = np.empty((2 * D,), dtype=np.float32)
    for p2 in range(NPAIR):
        wqk[:, 256 * p2:256 * p2 + 128] = w_qkv[:, 128 * p2:128 * (p2 + 1)]
        wqk[:, 256 * p2 + 128:256 * p2 + 256] = \
            w_qkv[:, D + 128 * p2:D + 128 * (p2 + 1)]
        bqk[256 * p2:256 * p2 + 128] = b_qkv[128 * p2:128 * (p2 + 1)]
        bqk[256 * p2 + 128:256 * p2 + 256] = \
            b_qkv[D + 128 * p2:D + 128 * (p2 + 1)]
    wv = np.zeros((D, VW), dtype=np.float16)
    bv = np.zeros((VW,), dtype=np.float32)
    for h in range(H):
        wv[:, 65 * h:65 * h + 64] = \
            w_qkv[:, 2 * D + 64 * h:2 * D + 64 * h + 64]
        bv[65 * h:65 * h + 64] = b_qkv[2 * D + 64 * h:2 * D + 64 * h + 64]
        bv[65 * h + 64] = 1.0
    return {
        "wqk": wqk,
        "bqk": bqk,
        "wv": wv,
        "bv": bv,
        "w_proj": np.asarray(inputs["w_proj"], dtype=np.float32).astype(np.float16),
        "b_proj": np.asarray(inputs["b_proj"], dtype=np.float32),
    }


def run(inputs: dict, trace: bool = False):
    """Build, compile and run on all 8 cores. Returns (out [B,N,D], results)."""
    nc = build_nc()
    x = np.asarray(inputs["x"], dtype=np.float32)
    shared = _prep_host(inputs)
    in_maps = [
        {"xT": np.ascontiguousarray(x[b].T.astype(np.float16)), **shared}
        for b in range(B)
    ]
    res = run_bass_kernel_spmd(nc, in_maps, list(range(B)), trace=trace)
    out = np.stack(
        [res.results[b]["out"].astype(np.float32) for b in range(B)], axis=0
    )
    return out, res


def kernel(x, w_qkv, b_qkv, w_proj, b_proj) -> np.ndarray:
    out, _ = run(
        {"x": x, "w_qkv": w_qkv, "b_qkv": b_qkv, "w_proj": w_proj,
         "b_proj": b_proj}
    )
    return out
